# revision 6
# baseline (speedup 1.0000x reference)
"""Trainium2 Bass kernel for the SOCS lithography simulator.

Reference math (per batch b):
    aerial = sum_k s_k * | cIFFT2( cFFT2(mask_b) * pad_center(kernels[k]) ) |^2
    resist = sigmoid(50*(aerial - 0.225));  printed = (resist > 0.5) = (aerial > 0.225)

The padded kernels live in the *frequency* domain with only a 35x35 window of
nonzero coefficients (rows/cols 494:529 of the centered spectrum), so every
field is band-limited to 35x35 frequencies and aerial (a sum of |field|^2) is
band-limited to 69x69.  That turns the whole thing into small dense matmuls:

    Mhat  = A @ x @ A.T          A = rows 494:529 of the centered DFT matrix   [35,35]  cplx
    G_k   = Mhat * (sqrt(s_k) * kernels[k])                                    [35,35]  cplx
    W_k   = G_k @ C.T            C = coarse (stride-8) inverse-DFT samples     [35,128] cplx
    Fc_k  = C @ W_k              == fields sampled on the 128x128 coarse grid  [128,128] cplx
    aer_c = sum_k |Fc_k|^2       exact coarse samples of aerial                [128,128] real
    aerial = U @ aer_c @ U.T     U real [1024,128] Dirichlet interpolation (exact)

Sharding: 8 cores; core c handles batch c//2 and output row-half c%2.
Each core runs stages 1-4 for its batch (cheap) and half of stage 5.
No collectives needed.

This file is self-contained: shapes/constants are hardcoded, no sibling imports.
"""

import math
import os

import numpy as np

N = 1024
B, K, HK = 4, 24, 35
PT = (N - HK) // 2          # 494
NC = 128                    # coarse grid
NF = 2 * HK - 1             # 69 product frequencies
DOSE = 1.0
RESIST_THRESHOLD = 0.225
RESIST_STEEPNESS = 50.0

F32 = None  # set after mybir import


# ---------------------------------------------------------------- host matrices
def _host_matrices():
    """Input-independent constant matrices (float32/complex64 kept in f64 here)."""
    u = np.arange(HK)[:, None]          # 0..34  (centered freq u-18)
    y = np.arange(N)[None, :]
    A = np.exp(-2j * np.pi * ((u + PT - N // 2) * (y - N // 2)) / N)  # [35,1024]
    # coarse inverse-DFT samples: C[m, u] = conj(A[u, 8m]) / N
    Cc = np.conj(A[:, ::8]).T / N                                     # [128,35]
    # U[y, m] = (1/128) * sum_{f=-34..34} exp(2pi i f (y-8m)/1024)  (real Dirichlet)
    f = np.arange(-(NF // 2), NF // 2 + 1)
    yy = np.arange(N)[:, None]
    mm = np.arange(NC)[None, :]
    ang = 2 * np.pi * (yy - 8 * mm) / N
    # sum of cos(f*ang) over f in [-34,34]  == Dirichlet kernel (vectorized, stable)
    U = np.ones((N, NC))
    for ff in range(1, NF // 2 + 1):
        U += 2.0 * np.cos(ff * ang)
    U /= NC

    atp = np.empty((N, 2 * HK), np.float32)          # [1024, 70]  A^T packed
    atp[:, :HK] = A.real.T
    atp[:, HK:] = A.imag.T
    ctr = np.ascontiguousarray(Cc.real.T, np.float32)   # Ctr[q, m] = Re C[m,q]  [35,128]
    cti = np.ascontiguousarray(Cc.imag.T, np.float32)
    ctp_top = np.concatenate([ctr, cti], axis=1)        # [35,256]
    ctp_bot = np.concatenate([-cti, ctr], axis=1)       # [35,256]
    ut = np.ascontiguousarray(U.T, np.float32)          # [128,1024]
    return atp, ctr, cti, ctp_top, ctp_bot, ut, U.astype(np.float32)


# ---------------------------------------------------------------- bass program
def _build_program():
    import concourse.bass as bass
    import concourse.mybir as mybir
    import concourse.tile as tile
    from concourse import bacc

    f32 = mybir.dt.float32
    AF = mybir.ActivationFunctionType
    ALU = mybir.AluOpType

    nc = bacc.Bacc("TRN2", target_bir_lowering=False, debug=False)

    x_d = nc.dram_tensor("x", [N, N], f32, kind="ExternalInput")
    atp_d = nc.dram_tensor("atp", [N, 2 * HK], f32, kind="ExternalInput")
    ktR_d = nc.dram_tensor("ktR", [HK, K * HK], f32, kind="ExternalInput")
    ktI_d = nc.dram_tensor("ktI", [HK, K * HK], f32, kind="ExternalInput")
    ctp_top_d = nc.dram_tensor("ctp_top", [HK, 2 * NC], f32, kind="ExternalInput")
    ctp_bot_d = nc.dram_tensor("ctp_bot", [HK, 2 * NC], f32, kind="ExternalInput")
    ctd_d = nc.dram_tensor("ctd", [3, HK, NC], f32, kind="ExternalInput")
    uht_d = nc.dram_tensor("uht", [NC, 512], f32, kind="ExternalInput")
    ut_d = nc.dram_tensor("ut", [NC, N], f32, kind="ExternalInput")

    aerial_d = nc.dram_tensor("aerial", [512, N], f32, kind="ExternalOutput")
    resist_d = nc.dram_tensor("resist", [512, N], f32, kind="ExternalOutput")
    printed_d = nc.dram_tensor("printed", [512, N], f32, kind="ExternalOutput")

    with tile.TileContext(nc) as tc:
        with (
            tc.tile_pool(name="const", bufs=1) as cpool,
            tc.tile_pool(name="xin", bufs=3) as xpool,
            tc.tile_pool(name="work", bufs=1) as wpool,
            tc.tile_pool(name="scratch", bufs=2) as spool,
            tc.tile_pool(name="sq", bufs=4) as sqpool,
            tc.tile_pool(name="outp", bufs=6) as opool,
        ):
            # ---- constants in ----
            atp_sb = cpool.tile([128, 8, 2 * HK], f32)
            for c in range(8):
                nc.sync.dma_start(atp_sb[:, c, :], atp_d[c * 128:(c + 1) * 128, :])
            ktR_sb = cpool.tile([HK, K * HK], f32)
            ktI_sb = cpool.tile([HK, K * HK], f32)
            nc.sync.dma_start(ktR_sb[:], ktR_d[:, :])
            nc.sync.dma_start(ktI_sb[:], ktI_d[:, :])
            ctp_top_sb = cpool.tile([HK, 2 * NC], f32)
            ctp_bot_sb = cpool.tile([HK, 2 * NC], f32)
            nc.sync.dma_start(ctp_top_sb[:], ctp_top_d[:, :])
            nc.sync.dma_start(ctp_bot_sb[:], ctp_bot_d[:, :])
            ct_r = cpool.tile([HK, NC], f32)
            ct_i = cpool.tile([HK, NC], f32)
            ct_in = cpool.tile([HK, NC], f32)
            nc.sync.dma_start(ct_r[:], ctd_d[0, :, :])
            nc.sync.dma_start(ct_i[:], ctd_d[1, :, :])
            nc.sync.dma_start(ct_in[:], ctd_d[2, :, :])
            uht_sb = cpool.tile([NC, 512], f32)
            nc.sync.dma_start(uht_sb[:], uht_d[:, :])
            ut_sb = cpool.tile([NC, N], f32)
            nc.sync.dma_start(ut_sb[:], ut_d[:, :])
            sig_bias = cpool.tile([128, 1], f32)
            nc.vector.memset(sig_bias[:], -RESIST_STEEPNESS * RESIST_THRESHOLD)

            p1t_sb = wpool.tile([128, 8, 2 * HK], f32)   # P1^T chunks [j-chunk][j, 70]

            # ---- stage 1: P1T[j,u] = sum_y x[y,j] * atp[y,u]  (70 = Re|Im of A) ----
            with tc.tile_pool(name="p1t_ps", bufs=8, space=bass.MemorySpace.PSUM) as p1ps:
                p1t_ps = [p1ps.tile([128, 2 * HK], f32, tag="p1t", name=f"p1t_ps{i}")
                          for i in range(8)]
                for yc in range(8):
                    x_sb = xpool.tile([128, N], f32)
                    nc.sync.dma_start(x_sb[:], x_d[yc * 128:(yc + 1) * 128, :])
                    for jc in range(8):
                        nc.tensor.matmul(
                            p1t_ps[jc][:, :],
                            x_sb[:, jc * 128:(jc + 1) * 128],
                            atp_sb[:, yc, :],
                            start=(yc == 0), stop=(yc == 7),
                        )
                for jc in range(8):
                    nc.scalar.copy(p1t_sb[:, jc, :], p1t_ps[jc][:, :])

            # ---- stage 1b: MhatT = A @ P1^T  (contract over j) ----
            mhat_r = wpool.tile([HK, HK], f32)
            mhat_i = wpool.tile([HK, HK], f32)
            with tc.tile_pool(name="m4_ps", bufs=2, space=bass.MemorySpace.PSUM) as m4ps:
                m4a = m4ps.tile([HK, 2 * HK], f32)   # Ar @ [P1rT | P1iT]
                m4b = m4ps.tile([HK, 2 * HK], f32)   # Ai @ [P1rT | P1iT]
                for jc in range(8):
                    nc.tensor.matmul(m4a[:, :], atp_sb[:, jc, 0:HK],
                                     p1t_sb[:, jc, :], start=(jc == 0), stop=(jc == 7))
                    nc.tensor.matmul(m4b[:, :], atp_sb[:, jc, HK:2 * HK],
                                     p1t_sb[:, jc, :], start=(jc == 0), stop=(jc == 7))
                # MhatT_r = ArP1r^T - AiP1i^T ; MhatT_i = ArP1i^T + AiP1r^T
                # (DVE may read at most one PSUM operand -> stage m4b via SBUF)
                m4b_sb = wpool.tile([HK, 2 * HK], f32)
                nc.scalar.copy(m4b_sb[:], m4b[:])
                nc.vector.tensor_sub(mhat_r[:], m4a[:, 0:HK], m4b_sb[:, HK:2 * HK])
                nc.vector.tensor_add(mhat_i[:], m4a[:, HK:2 * HK], m4b_sb[:, 0:HK])

            # ---- stage 2a: Gt blocks for all k:  Gt = MhatT .* Kt  (complex) ----
            gtr = wpool.tile([HK, K * HK], f32)
            gti = wpool.tile([HK, K * HK], f32)
            mr_b = mhat_r[:].unsqueeze(1).broadcast_to([HK, K, HK])
            mi_b = mhat_i[:].unsqueeze(1).broadcast_to([HK, K, HK])
            s1 = spool.tile([HK, K * HK], f32, tag="s2a")
            s2 = spool.tile([HK, K * HK], f32, tag="s2a")
            ktR3 = ktR_sb[:].rearrange("q (k p) -> q k p", k=K)
            ktI3 = ktI_sb[:].rearrange("q (k p) -> q k p", k=K)
            s13 = s1[:].rearrange("q (k p) -> q k p", k=K)
            s23 = s2[:].rearrange("q (k p) -> q k p", k=K)
            nc.vector.tensor_mul(s13, mr_b, ktR3)
            nc.vector.tensor_mul(s23, mi_b, ktI3)
            nc.vector.tensor_sub(gtr[:], s1[:], s2[:])
            s3 = spool.tile([HK, K * HK], f32, tag="s2a")
            s4 = spool.tile([HK, K * HK], f32, tag="s2a")
            s33 = s3[:].rearrange("q (k p) -> q k p", k=K)
            s43 = s4[:].rearrange("q (k p) -> q k p", k=K)
            nc.vector.tensor_mul(s33, mr_b, ktI3)
            nc.vector.tensor_mul(s43, mi_b, ktR3)
            nc.vector.tensor_add(gti[:], s3[:], s4[:])

            # ---- stage 2c: W_k = G_k @ C^T  -> [Wr | Wi] [35, 256] per k ----
            w_all = wpool.tile([HK, K * 2 * NC], f32)     # [35, 6144]
            with (
                tc.tile_pool(name="w_ps", bufs=2, space=bass.MemorySpace.PSUM) as wps,
                tc.tile_pool(name="f_ps", bufs=2, space=bass.MemorySpace.PSUM) as fps,
            ):
                for g4 in range(6):                        # 4 kernels per psum tile
                    wp = wps.tile([HK, 4 * 2 * NC], f32)   # [35, 1024]
                    for j in range(4):
                        k = 4 * g4 + j
                        nc.tensor.matmul(wp[:, j * 256:(j + 1) * 256],
                                         gtr[:, k * HK:(k + 1) * HK],
                                         ctp_top_sb[:], start=True, stop=False)
                        nc.tensor.matmul(wp[:, j * 256:(j + 1) * 256],
                                         gti[:, k * HK:(k + 1) * HK],
                                         ctp_bot_sb[:], start=False, stop=True)
                    if g4 % 2 == 0:
                        nc.scalar.copy(w_all[:, g4 * 1024:(g4 + 1) * 1024], wp[:])
                    else:
                        nc.vector.tensor_copy(w_all[:, g4 * 1024:(g4 + 1) * 1024], wp[:])

                # ---- stage 2d: Fc groups + |.|^2 accumulation ----
                aer4 = wpool.tile([128, 512], f32)         # 4 k-blocks of [128,128]
                w4 = w_all[:].rearrange("q (k c m) -> q k c m", k=K, c=2)
                for g in range(6):
                    fp = fps.tile([128, 1024], f32)        # [Fr | Fi]
                    wr = w4[:, 4 * g:4 * (g + 1), 0, :]    # [35, 4, 128]
                    wi = w4[:, 4 * g:4 * (g + 1), 1, :]
                    nc.tensor.matmul(fp[:, 0:512], ct_r[:], wr, start=True, stop=False)
                    nc.tensor.matmul(fp[:, 0:512], ct_in[:], wi, start=False, stop=True)
                    nc.tensor.matmul(fp[:, 512:1024], ct_i[:], wr, start=True, stop=False)
                    nc.tensor.matmul(fp[:, 512:1024], ct_r[:], wi, start=False, stop=True)
                    sq_r = sqpool.tile([128, 512], f32, tag="sq")
                    sq_i = sqpool.tile([128, 512], f32, tag="sq")
                    nc.scalar.activation(sq_r[:], fp[:, 0:512], AF.Square)
                    nc.scalar.activation(sq_i[:], fp[:, 512:1024], AF.Square)
                    if g == 0:
                        nc.vector.tensor_add(aer4[:], sq_r[:], sq_i[:])
                    else:
                        nc.vector.tensor_add(aer4[:], aer4[:], sq_r[:])
                        nc.vector.tensor_add(aer4[:], aer4[:], sq_i[:])

            # fold the 4 k-blocks -> aer_c [128, 128]
            u1 = wpool.tile([128, 256], f32)
            aer_c = wpool.tile([128, 128], f32)
            nc.vector.tensor_add(u1[:], aer4[:, 0:256], aer4[:, 256:512])
            nc.vector.tensor_add(aer_c[:], u1[:, 0:128], u1[:, 128:256])

            # ---- stage 5: aerial_half = U_h @ aer_c @ U^T ----
            z_sb = wpool.tile([128, 512], f32)
            with tc.tile_pool(name="z_ps", bufs=1, space=bass.MemorySpace.PSUM) as zps:
                zp = zps.tile([128, 512], f32)
                nc.tensor.matmul(zp[:], aer_c[:], uht_sb[:], start=True, stop=True)
                nc.scalar.copy(z_sb[:], zp[:])

            with tc.tile_pool(name="a_ps", bufs=2, space=bass.MemorySpace.PSUM) as aps:
                for t in range(4):
                    ap_t = aps.tile([128, N], f32)
                    for j in range(2):
                        nc.tensor.matmul(ap_t[:, j * 512:(j + 1) * 512],
                                         z_sb[:, t * 128:(t + 1) * 128],
                                         ut_sb[:, j * 512:(j + 1) * 512],
                                         start=True, stop=True)
                    aer_sb = opool.tile([128, N], f32, tag="out")
                    res_sb = opool.tile([128, N], f32, tag="out")
                    prn_sb = opool.tile([128, N], f32, tag="out")
                    nc.scalar.copy(aer_sb[:], ap_t[:])
                    nc.scalar.activation(res_sb[:], ap_t[:], AF.Sigmoid,
                                         bias=sig_bias[:],
                                         scale=RESIST_STEEPNESS)
                    nc.vector.tensor_scalar(prn_sb[:], ap_t[:], RESIST_THRESHOLD, None,
                                            op0=ALU.is_gt)
                    nc.sync.dma_start(aerial_d[t * 128:(t + 1) * 128, :], aer_sb[:])
                    nc.sync.dma_start(resist_d[t * 128:(t + 1) * 128, :], res_sb[:])
                    nc.sync.dma_start(printed_d[t * 128:(t + 1) * 128, :], prn_sb[:])

    nc.compile()
    return nc


_CACHE = {}


def _get_program():
    if "nc" not in _CACHE:
        _CACHE["nc"] = _build_program()
    return _CACHE["nc"]


# ---------------------------------------------------------------- entry point
def kernel(mask, kernels, kernels_ct, scales):
    """Full inputs in, full outputs out.  Shards over 8 NeuronCores internally."""
    from concourse.bass_utils import run_bass_kernel_spmd

    mask = np.ascontiguousarray(np.asarray(mask, np.float32))
    kernels = np.asarray(kernels, np.complex64)
    scales = np.asarray(scales, np.float32)

    atp, ctr, cti, ctp_top, ctp_bot, ut, U = _host_matrices()
    ctd = np.stack([ctr, cti, -cti])                     # [3,35,128]

    # runtime-scaled, transposed kernels: Kt[q, k*35+p] = Re/Im(kernels[k][p,q]*sqrt(s_k))
    kers = kernels * np.sqrt(scales.astype(np.float64)).astype(np.float32)[:, None, None]
    ktR = np.ascontiguousarray(
        kers.real.transpose(2, 0, 1).reshape(HK, K * HK), np.float32)
    ktI = np.ascontiguousarray(
        kers.imag.transpose(2, 0, 1).reshape(HK, K * HK), np.float32)

    uh = [np.ascontiguousarray(U[h * 512:(h + 1) * 512, :].T) for h in range(2)]

    nc = _get_program()
    in_maps = []
    for c in range(8):
        b, h = c // 2, c % 2
        in_maps.append({
            "x": mask[b],
            "atp": atp,
            "ktR": ktR,
            "ktI": ktI,
            "ctp_top": ctp_top,
            "ctp_bot": ctp_bot,
            "ctd": ctd,
            "uht": uh[h],
            "ut": ut,
        })

    trace = bool(int(os.environ.get("BASS_KERNEL_TRACE", "0")))
    res = run_bass_kernel_spmd(nc, in_maps, core_ids=list(range(8)), trace=trace)
    _CACHE["last_results"] = res

    aerial = np.empty((B, N, N), np.float32)
    resist = np.empty((B, N, N), np.float32)
    printed = np.empty((B, N, N), np.float32)
    for c in range(8):
        b, h = c // 2, c % 2
        rows = slice(h * 512, (h + 1) * 512)
        aerial[b, rows, :] = res.results[c]["aerial"]
        resist[b, rows, :] = res.results[c]["resist"]
        printed[b, rows, :] = res.results[c]["printed"]
    return aerial, resist, printed


# revision 8
# speedup vs baseline: 2.1112x; 2.1112x over previous
"""Trainium2 Bass kernel for the SOCS lithography simulator.

Reference math (per batch b):
    aerial = sum_k s_k * | cIFFT2( cFFT2(mask_b) * pad_center(kernels[k]) ) |^2
    resist = sigmoid(50*(aerial - 0.225));  printed = (resist > 0.5) = (aerial > 0.225)

The padded kernels live in the *frequency* domain with only a 35x35 window of
nonzero coefficients (rows/cols 494:529 of the centered spectrum), so every
field is band-limited to 35x35 frequencies and aerial (a sum of |field|^2) is
band-limited to 69x69.  That turns the whole thing into small dense matmuls:

    Mhat  = A @ x @ A.T          A = rows 494:529 of the centered DFT matrix   [35,35]  cplx
    G_k   = Mhat * (sqrt(s_k) * kernels[k])                                    [35,35]  cplx
    W_k   = G_k @ C.T            C = coarse (stride-8) inverse-DFT samples     [35,128] cplx
    Fc_k  = C @ W_k              == fields sampled on the 128x128 coarse grid  [128,128] cplx
    aer_c = sum_k |Fc_k|^2       exact coarse samples of aerial                [128,128] real
    aerial = U @ aer_c @ U.T     U real [1024,128] Dirichlet interpolation (exact)

Precision: stages 1-2 in bf16 (verified ~9e-4 rel l2 end-to-end), final
interpolation (stage 5) in f32r.

Sharding: 8 cores; core c handles batch c//2 and output row-half c%2.
Each core runs stages 1-4 for its batch (cheap) and half of stage 5.
No collectives needed.

This file is self-contained: shapes/constants are hardcoded, no sibling imports.
"""

import os

import numpy as np

N = 1024
B, K, HK = 4, 24, 35
PT = (N - HK) // 2          # 494
NC = 128                    # coarse grid
NF = 2 * HK - 1             # 69 product frequencies
DOSE = 1.0
RESIST_THRESHOLD = 0.225
RESIST_STEEPNESS = 50.0


# ---------------------------------------------------------------- host matrices
def _host_matrices():
    """Input-independent constant matrices."""
    u = np.arange(HK)[:, None]          # 0..34  (centered freq u-18)
    y = np.arange(N)[None, :]
    A = np.exp(-2j * np.pi * ((u + PT - N // 2) * (y - N // 2)) / N)  # [35,1024]
    # coarse inverse-DFT samples: C[m, u] = conj(A[u, 8m]) / N
    Cc = np.conj(A[:, ::8]).T / N                                     # [128,35]
    # U[y, m] = (1/128) * sum_{f=-34..34} exp(2pi i f (y-8m)/1024)  (real Dirichlet)
    yy = np.arange(N)[:, None]
    mm = np.arange(NC)[None, :]
    ang = 2 * np.pi * (yy - 8 * mm) / N
    U = np.ones((N, NC))
    for ff in range(1, NF // 2 + 1):
        U += 2.0 * np.cos(ff * ang)
    U /= NC

    atp = np.empty((N, 2 * HK), np.float32)          # [1024, 70]  A^T packed
    atp[:, :HK] = A.real.T
    atp[:, HK:] = A.imag.T
    ctr = np.ascontiguousarray(Cc.real.T, np.float32)   # Ctr[q, m] = Re C[m,q]  [35,128]
    cti = np.ascontiguousarray(Cc.imag.T, np.float32)
    ctp_top = np.concatenate([ctr, cti], axis=1)        # [35,256]
    ctp_bot = np.concatenate([-cti, ctr], axis=1)       # [35,256]
    ut = np.ascontiguousarray(U.T, np.float32)          # [128,1024]
    return atp, ctr, cti, ctp_top, ctp_bot, ut, U.astype(np.float32)


# ---------------------------------------------------------------- bass program
def _build_program():
    import concourse.bass as bass
    import concourse.mybir as mybir
    import concourse.tile as tile
    from concourse import bacc

    f32 = mybir.dt.float32
    f32r = mybir.dt.float32r
    bf16 = mybir.dt.bfloat16
    AF = mybir.ActivationFunctionType
    ALU = mybir.AluOpType

    nc = bacc.Bacc("TRN2", target_bir_lowering=False, debug=False)

    x_d = nc.dram_tensor("x", [N, N], bf16, kind="ExternalInput")
    atp_d = nc.dram_tensor("atp", [N, 2 * HK], bf16, kind="ExternalInput")
    # kconst = [ktR | ktI]  [35, 1680]
    kconst_d = nc.dram_tensor("kconst", [HK, 2 * K * HK], bf16, kind="ExternalInput")
    # cconst = [ctp_top | ctp_bot | ctr | cti | -cti]  [35, 896]
    cconst_d = nc.dram_tensor("cconst", [HK, 896], bf16, kind="ExternalInput")
    # uconst = [uht | ut]  [128, 1536] f32
    uconst_d = nc.dram_tensor("uconst", [NC, 1536], f32r, kind="ExternalInput")

    aerial_d = nc.dram_tensor("aerial", [512, N], f32, kind="ExternalOutput")
    resist_d = nc.dram_tensor("resist", [512, N], f32, kind="ExternalOutput")
    printed_d = nc.dram_tensor("printed", [512, N], f32, kind="ExternalOutput")

    with tile.TileContext(nc) as tc:
        with (
            tc.tile_pool(name="const", bufs=1) as cpool,
            tc.tile_pool(name="xin", bufs=4) as xpool,
            tc.tile_pool(name="work", bufs=1) as wpool,
            tc.tile_pool(name="scratch", bufs=2) as spool,
            tc.tile_pool(name="sq", bufs=4) as sqpool,
            tc.tile_pool(name="outp", bufs=6) as opool,
        ):
            # ---- x chunks first (sync queue), consts on gpsimd queue ----
            x_sb = [xpool.tile([128, N], bf16, tag="x", name=f"x{i}") for i in range(8)]
            for yc in range(8):
                nc.sync.dma_start(x_sb[yc][:], x_d[yc * 128:(yc + 1) * 128, :])

            atp_sb = cpool.tile([128, 8, 2 * HK], bf16)
            nc.gpsimd.dma_start(
                atp_sb[:],
                atp_d.ap().rearrange("(c p) u -> p c u", p=128))
            kconst_sb = cpool.tile([HK, 2 * K * HK], bf16)
            nc.gpsimd.dma_start(kconst_sb[:], kconst_d[:, :])
            cconst_sb = cpool.tile([HK, 896], bf16)
            nc.gpsimd.dma_start(cconst_sb[:], cconst_d[:, :])
            uconst_sb = cpool.tile([NC, 1536], f32r)
            nc.gpsimd.dma_start(uconst_sb[:], uconst_d[:, :])
            sig_bias = cpool.tile([128, 1], f32)
            nc.vector.memset(sig_bias[:], -RESIST_STEEPNESS * RESIST_THRESHOLD)

            ktR3 = kconst_sb[:, 0:K * HK].rearrange("q (k p) -> q k p", k=K)
            ktI3 = kconst_sb[:, K * HK:2 * K * HK].rearrange("q (k p) -> q k p", k=K)
            ctp_top = cconst_sb[:, 0:256]
            ctp_bot = cconst_sb[:, 256:512]
            ct_r = cconst_sb[:, 512:640]
            ct_i = cconst_sb[:, 640:768]
            ct_in = cconst_sb[:, 768:896]
            uht_sb = uconst_sb[:, 0:512]
            ut_sb = uconst_sb[:, 512:1536]

            p1t_sb = wpool.tile([128, 8, 2 * HK], bf16)   # P1^T chunks

            # ---- stage 1: P1T[j,u] = sum_y x[y,j] * atp[y,u] ----
            with tc.tile_pool(name="p1t_ps", bufs=8, space=bass.MemorySpace.PSUM) as p1ps:
                p1t_ps = [p1ps.tile([128, 2 * HK], f32, tag="p1t", name=f"p1t_ps{i}")
                          for i in range(8)]
                for yc in range(8):
                    for jc in range(8):
                        nc.tensor.matmul(
                            p1t_ps[jc][:, :],
                            x_sb[yc][:, jc * 128:(jc + 1) * 128],
                            atp_sb[:, yc, :],
                            start=(yc == 0), stop=(yc == 7),
                        )
                for jc in range(8):
                    nc.scalar.copy(p1t_sb[:, jc, :], p1t_ps[jc][:, :])

            # ---- stage 1b: MhatT = A @ P1^T  (contract over j) ----
            mhat_r = wpool.tile([HK, HK], f32)
            mhat_i = wpool.tile([HK, HK], f32)
            with tc.tile_pool(name="m4_ps", bufs=2, space=bass.MemorySpace.PSUM) as m4ps:
                m4a = m4ps.tile([HK, 2 * HK], f32)   # Ar @ [P1rT | P1iT]
                m4b = m4ps.tile([HK, 2 * HK], f32)   # Ai @ [P1rT | P1iT]
                for jc in range(8):
                    nc.tensor.matmul(m4a[:, :], atp_sb[:, jc, 0:HK],
                                     p1t_sb[:, jc, :], start=(jc == 0), stop=(jc == 7))
                    nc.tensor.matmul(m4b[:, :], atp_sb[:, jc, HK:2 * HK],
                                     p1t_sb[:, jc, :], start=(jc == 0), stop=(jc == 7))
                # MhatT_r = ArP1r^T - AiP1i^T ; MhatT_i = ArP1i^T + AiP1r^T
                # (DVE may read at most one PSUM operand -> stage m4b via SBUF)
                m4b_sb = wpool.tile([HK, 2 * HK], f32)
                nc.scalar.copy(m4b_sb[:], m4b[:])
                nc.vector.tensor_sub(mhat_r[:], m4a[:, 0:HK], m4b_sb[:, HK:2 * HK])
                nc.vector.tensor_add(mhat_i[:], m4a[:, HK:2 * HK], m4b_sb[:, 0:HK])

            # ---- stage 2a: Gt blocks for all k:  Gt = MhatT .* Kt  (complex) ----
            gtr = wpool.tile([HK, K * HK], bf16)
            gti = wpool.tile([HK, K * HK], bf16)
            mr_b = mhat_r[:].unsqueeze(1).broadcast_to([HK, K, HK])
            mi_b = mhat_i[:].unsqueeze(1).broadcast_to([HK, K, HK])
            s1 = spool.tile([HK, K * HK], f32, tag="s2a", name="s1")
            s2 = spool.tile([HK, K * HK], f32, tag="s2a", name="s2")
            s3 = spool.tile([HK, K * HK], f32, tag="s2b", name="s3")
            s4 = spool.tile([HK, K * HK], f32, tag="s2b", name="s4")
            r3 = lambda t: t[:].rearrange("q (k p) -> q k p", k=K)
            # DVE computes gtr while GpSimd computes gti (independent chains)
            nc.vector.tensor_mul(r3(s1), mr_b, ktR3)
            nc.vector.tensor_mul(r3(s2), mi_b, ktI3)
            nc.vector.tensor_sub(gtr[:], s1[:], s2[:])
            nc.gpsimd.tensor_mul(r3(s3), mr_b, ktI3)
            nc.gpsimd.tensor_mul(r3(s4), mi_b, ktR3)
            nc.gpsimd.tensor_add(gti[:], s3[:], s4[:])

            # ---- stage 2c: W_k = G_k @ C^T  -> [Wr | Wi] [35, 256] per k ----
            w_all = wpool.tile([HK, K * 2 * NC], bf16)     # [35, 6144]
            with (
                tc.tile_pool(name="w_ps", bufs=2, space=bass.MemorySpace.PSUM) as wps,
                tc.tile_pool(name="f_ps", bufs=2, space=bass.MemorySpace.PSUM) as fps,
            ):
                for g4 in range(6):                        # 4 kernels per psum tile
                    wp = wps.tile([HK, 4 * 2 * NC], f32)   # [35, 1024]
                    for j in range(4):
                        k = 4 * g4 + j
                        nc.tensor.matmul(wp[:, j * 256:(j + 1) * 256],
                                         gtr[:, k * HK:(k + 1) * HK],
                                         ctp_top, start=True, stop=False)
                        nc.tensor.matmul(wp[:, j * 256:(j + 1) * 256],
                                         gti[:, k * HK:(k + 1) * HK],
                                         ctp_bot, start=False, stop=True)
                    nc.scalar.copy(w_all[:, g4 * 1024:(g4 + 1) * 1024], wp[:])

                # ---- stage 2d: Fc groups + |.|^2 accumulation ----
                aer4 = wpool.tile([128, 512], f32)         # 4 k-blocks of [128,128]
                w4 = w_all[:].rearrange("q (k c m) -> q k c m", k=K, c=2)
                for g in range(6):
                    fp = fps.tile([128, 1024], f32)        # [Fr | Fi]
                    wr = w4[:, 4 * g:4 * (g + 1), 0, :]    # [35, 4, 128]
                    wi = w4[:, 4 * g:4 * (g + 1), 1, :]
                    nc.tensor.matmul(fp[:, 0:512], ct_r, wr, start=True, stop=False)
                    nc.tensor.matmul(fp[:, 0:512], ct_in, wi, start=False, stop=True)
                    nc.tensor.matmul(fp[:, 512:1024], ct_i, wr, start=True, stop=False)
                    nc.tensor.matmul(fp[:, 512:1024], ct_r, wi, start=False, stop=True)
                    sq_r = sqpool.tile([128, 512], f32, tag="sq", name="sq_r")
                    sq_i = sqpool.tile([128, 512], f32, tag="sq", name="sq_i")
                    nc.scalar.activation(sq_r[:], fp[:, 0:512], AF.Square)
                    nc.scalar.activation(sq_i[:], fp[:, 512:1024], AF.Square)
                    if g == 0:
                        nc.vector.tensor_add(aer4[:], sq_r[:], sq_i[:])
                    else:
                        eng = nc.vector if g % 2 == 1 else nc.gpsimd
                        eng.tensor_add(aer4[:], aer4[:], sq_r[:])
                        eng.tensor_add(aer4[:], aer4[:], sq_i[:])

            # fold the 4 k-blocks -> aer_c [128, 128]
            u1 = wpool.tile([128, 256], f32)
            aer_c = wpool.tile([128, 128], f32r)
            nc.vector.tensor_add(u1[:], aer4[:, 0:256], aer4[:, 256:512])
            nc.vector.tensor_add(aer_c[:], u1[:, 0:128], u1[:, 128:256])

            # ---- stage 5: aerial_half = U_h @ aer_c @ U^T  (f32r matmuls) ----
            z_sb = wpool.tile([128, 512], f32r)
            with tc.tile_pool(name="z_ps", bufs=1, space=bass.MemorySpace.PSUM) as zps:
                zp = zps.tile([128, 512], f32)
                nc.tensor.matmul(zp[:], aer_c[:], uht_sb, start=True, stop=True)
                nc.scalar.copy(z_sb[:], zp[:])

            with tc.tile_pool(name="a_ps", bufs=2, space=bass.MemorySpace.PSUM) as aps:
                for t in range(4):
                    ap_t = aps.tile([128, N], f32)
                    for j in range(2):
                        nc.tensor.matmul(ap_t[:, j * 512:(j + 1) * 512],
                                         z_sb[:, t * 128:(t + 1) * 128],
                                         ut_sb[:, j * 512:(j + 1) * 512],
                                         start=True, stop=True)
                    aer_sb = opool.tile([128, N], f32, tag="out", name="aer_sb")
                    res_sb = opool.tile([128, N], f32, tag="out", name="res_sb")
                    prn_sb = opool.tile([128, N], f32, tag="out", name="prn_sb")
                    nc.scalar.copy(aer_sb[:], ap_t[:])
                    nc.scalar.activation(res_sb[:], aer_sb[:], AF.Sigmoid,
                                         bias=sig_bias[:],
                                         scale=RESIST_STEEPNESS)
                    nc.vector.tensor_scalar(prn_sb[:], aer_sb[:], RESIST_THRESHOLD, None,
                                            op0=ALU.is_gt)
                    nc.sync.dma_start(aerial_d[t * 128:(t + 1) * 128, :], aer_sb[:])
                    nc.sync.dma_start(resist_d[t * 128:(t + 1) * 128, :], res_sb[:])
                    nc.sync.dma_start(printed_d[t * 128:(t + 1) * 128, :], prn_sb[:])

    nc.compile()
    return nc


_CACHE = {}


def _get_program():
    if "nc" not in _CACHE:
        _CACHE["nc"] = _build_program()
    return _CACHE["nc"]


def _prep_inputs(mask, kernels, scales):
    import ml_dtypes
    bf = ml_dtypes.bfloat16

    atp, ctr, cti, ctp_top, ctp_bot, ut, U = _host_matrices()

    kers = kernels.astype(np.complex128) * np.sqrt(scales.astype(np.float64))[:, None, None]
    ktR = np.ascontiguousarray(
        kers.real.astype(np.float32).transpose(2, 0, 1).reshape(HK, K * HK))
    ktI = np.ascontiguousarray(
        kers.imag.astype(np.float32).transpose(2, 0, 1).reshape(HK, K * HK))
    kconst = np.concatenate([ktR, ktI], axis=1).astype(bf)
    cconst = np.concatenate([ctp_top, ctp_bot, ctr, cti, -cti], axis=1).astype(bf)
    uh = [np.ascontiguousarray(U[h * 512:(h + 1) * 512, :].T) for h in range(2)]
    uconst = [np.concatenate([uh[h], ut], axis=1).astype(np.float32) for h in range(2)]
    atp_bf = atp.astype(bf)
    mask_bf = np.asarray(mask, np.float32).astype(bf)
    return mask_bf, atp_bf, kconst, cconst, uconst


# ---------------------------------------------------------------- entry point
def kernel(mask, kernels, kernels_ct, scales):
    """Full inputs in, full outputs out.  Shards over 8 NeuronCores internally."""
    from concourse.bass_utils import run_bass_kernel_spmd

    kernels = np.asarray(kernels, np.complex64)
    scales = np.asarray(scales, np.float32)
    mask_bf, atp_bf, kconst, cconst, uconst = _prep_inputs(mask, kernels, scales)

    nc = _get_program()
    in_maps = []
    for c in range(8):
        b, h = c // 2, c % 2
        in_maps.append({
            "x": mask_bf[b],
            "atp": atp_bf,
            "kconst": kconst,
            "cconst": cconst,
            "uconst": uconst[h],
        })

    trace = bool(int(os.environ.get("BASS_KERNEL_TRACE", "0")))
    res = run_bass_kernel_spmd(nc, in_maps, core_ids=list(range(8)), trace=trace)
    _CACHE["last_results"] = res

    aerial = np.empty((B, N, N), np.float32)
    resist = np.empty((B, N, N), np.float32)
    printed = np.empty((B, N, N), np.float32)
    for c in range(8):
        b, h = c // 2, c % 2
        rows = slice(h * 512, (h + 1) * 512)
        aerial[b, rows, :] = res.results[c]["aerial"]
        resist[b, rows, :] = res.results[c]["resist"]
        printed[b, rows, :] = res.results[c]["printed"]
    return aerial, resist, printed
